# revision 18
# baseline (speedup 1.0000x reference)
"""Trainium2 Bass kernel for nn_BasicBlock_HMU (two HMU layers + sync BN + residual).

Sharding: data-parallel over batch (8 cores x 512 rows); params replicated.
BN batch statistics are all-gathered across the 8 cores (sync BN).

v4 — builds on v3.1 (n-on-partitions, fp8e4m3 DoubleRow, full error
compensation) with:
  - pass-3 weight reuse: the third recovery pass v832*(m-m8)*32 is replaced
    by v8*fp8(m-m8) — identical accuracy in emulation (6.28e-3 vs 6.24e-3)
    and drops the third weight stream: 9 blocks/n-tile instead of 13
    (18.8MB instead of 27.2MB of weights per core).
  - split-half sync BN: stats for n-tiles 0-3 are AllGathered at mid-sweep
    and finalized/normalized while tiles 4-7 still sweep; only the second
    half's collective + finalize sits on the critical path.
  - rsqrt via exp(-0.5*ln(w)): Ln/Exp/Square/Copy all live in one ACT
    table set, so no LoadActFuncSet thrash (Sqrt's set excludes Exp).
  - k-groups first, q-group last per n-tile: shortens the last-tile
    combine tail (t1=lam*q+s01 -> qf -> exp) and moves the |h|^2-row
    dependency off the L2 start; hsq matmuls for tiles 4-7 are emitted
    after L2-j0's k-groups so PE order never blocks on late converts.
  - L2 normalize+residual in place in e_all, drained in 3 merged DMAs.
  - startup: per-k first-tile weight DMAs interleaved with the x streams;
    distinct PE warm-up matmuls hand off to the sweep at full clock.
"""

import numpy as np
import ml_dtypes

import concourse.bacc as bacc
import concourse.mybir as mybir
import concourse.tile as tile

try:
    from concourse.bass_utils import run_bass_kernel_spmd
except ImportError:  # pragma: no cover
    from bass_utils import run_bass_kernel_spmd

# All ACT functions this kernel uses (Exp, Square, Ln, Copy/Identity) live in
# the single table set natural_log_exp_and_others. The default chooser mixes
# sets (Exp/Square from one, Ln from another), inserting 1.3us
# LoadActFuncSet switches on the critical path. Restrict it to the one set
# that covers everything (positions preserved so act_func_set_id still
# indexes act_info.json).
_ACT_SET = "natural_log_exp_and_others"
_orig_gat = bacc.get_activation_tables


def _gat_single(arch):
    tabs = _orig_gat(arch)
    if _ACT_SET in tabs:
        tabs = {k: (v if k == _ACT_SET else set()) for k, v in tabs.items()}
    return tabs


bacc.get_activation_tables = _gat_single

F32 = mybir.dt.float32
BF16 = mybir.dt.bfloat16
FP8 = mybir.dt.float8e4
Alu = mybir.AluOpType
Act = mybir.ActivationFunctionType
DR = mybir.MatmulPerfMode.DoubleRow
BF = ml_dtypes.bfloat16
F8 = mybir.dt.np(mybir.dt.float8e4)

N_CORES = 8
B, D, N, K = 4096, 1024, 1024, 4
BS = B // N_CORES          # 512 rows per core
NT = N // 128              # 8 n-tiles per layer
CH = D // 128              # 8 contraction chunks (4 DoubleRow pairs)
NBLK = 1 + 2 * K           # mu + (v8, vr8) x4 blocks per n-tile group
GW = NBLK * 1024           # packed group width (9216)
WCOL = NT * GW             # packed weight columns per layer
BN_EPS = 1e-5
C1 = 1024.0 / 3.0          # host-side shift of the |x|^2 row

_CACHE = {}
_MARKERS = []


def _build_nc(reps=1, loop_reps=0, collectives=True):
    nc = bacc.Bacc("TRN2", target_bir_lowering=False, debug=False,
                   num_devices=N_CORES)
    _MARKERS.clear()

    def mark(label):
        _MARKERS.append((label, nc._state.next_id()))

    x8_s = nc.dram_tensor("x8_s", [128, CH * BS], FP8, kind="ExternalInput").ap()
    xlo_s = nc.dram_tensor("xlo_s", [128, CH * BS], FP8, kind="ExternalInput").ap()
    xr_s = nc.dram_tensor("xr_s", [128, CH * BS], FP8, kind="ExternalInput").ap()
    xres_s = nc.dram_tensor("xres_s", [128, CH * BS], BF16, kind="ExternalInput").ap()
    sqk1_s = nc.dram_tensor("sqk1_s", [1, 2 * BS + 256], FP8,
                            kind="ExternalInput").ap()
    W1p = nc.dram_tensor("W1p", [128, WCOL], FP8, kind="ExternalInput").ap()
    W2p = nc.dram_tensor("W2p", [128, WCOL], FP8, kind="ExternalInput").ap()
    cst_s = nc.dram_tensor("cst_s", [128, 128], F32, kind="ExternalInput").ap()
    outT = nc.dram_tensor("outT", [N, BS], F32, kind="ExternalOutput").ap()

    def dr(ap):
        return ap.rearrange("p (two m) -> p two m", two=2)

    with tile.TileContext(nc) as tc:
        with (
            tc.tile_pool(name="const", bufs=1) as constp,
            tc.tile_pool(name="big", bufs=1) as bigp,
            tc.tile_pool(name="wp", bufs=4) as wp,
            tc.tile_pool(name="scr", bufs=2) as scr,
            tc.tile_pool(name="rowp", bufs=1) as rowp,
            tc.tile_pool(name="fin", bufs=2) as finp,
            tc.tile_pool(name="pq", bufs=2, space="PSUM") as pq,
            tc.tile_pool(name="pp", bufs=4, space="PSUM") as pp,
            tc.tile_pool(name="ph", bufs=1, space="PSUM") as php,
            tc.tile_pool(name="dram", bufs=2, space="DRAM") as dramp,
        ):
            # ---- constants (loaded once, shared across reps) ----
            sqk1 = constp.tile([1, 2 * BS + 256], FP8)
            nc.scalar.dma_start(sqk1[:], sqk1_s)
            cst = constp.tile([128, 128], F32)
            nc.scalar.dma_start(cst[:], cst_s)
            lamc = cst[:, 0:16]
            cexp = cst[:, 16:32]
            cv = cst[:, 32:96]
            gb = cst[:, 96:128]
            onesc = constp.tile([128, 1], FP8)
            nc.gpsimd.memset(onesc[:], 1.0)
            c1eps = constp.tile([128, 1], F32)
            nc.gpsimd.memset(c1eps[:], 1.0 + BN_EPS)
            wrow = constp.tile([1, 128], F32)
            nc.gpsimd.memset(wrow[:], 1.0)

            def body():
                x8 = bigp.tile([128, CH * BS], FP8, tag="x8")
                xlo = bigp.tile([128, CH * BS], FP8, tag="xlo")
                xr = bigp.tile([128, CH * BS], FP8, tag="xr")
                xres = bigp.tile([128, CH * BS], BF16, tag="xres")
                h8 = bigp.tile([128, NT * BS], FP8, tag="h8")
                h832 = bigp.tile([128, NT * BS], FP8, tag="h832")
                hr = bigp.tile([128, NT * BS], FP8, tag="hr")
                hsqrow = rowp.tile([1, 2 * BS], FP8, tag="hsqrow")
                trash = php.tile([1, BS], F32, tag="trash")
                hsqp = php.tile([1, BS], F32, tag="hsq")

                wts = {}
                hh_pend = []

                def load_wt(L, j):
                    wt = wp.tile([128, GW], FP8, tag="w")
                    Wp = (W1p, W2p)[L]
                    b0 = j * GW
                    nc.sync.dma_start(wt[:, 0:1024], Wp[:, b0:b0 + 1024])
                    q = (nc.gpsimd, nc.scalar)[j % 2]
                    q.dma_start(wt[:, 1024:GW], Wp[:, b0 + 1024:b0 + GW])
                    wts[(L, j)] = wt

                # ---- startup: first tile's weights split per-k and
                # interleaved with the x streams so k0 can start earliest ----
                wt0 = wp.tile([128, GW], FP8, tag="w")
                nc.sync.dma_start(wt0[:, 0:1024], W1p[:, 0:1024])
                nc.scalar.dma_start(x8[:], x8_s)
                nc.gpsimd.dma_start(wt0[:, 1024:3072], W1p[:, 1024:3072])
                nc.scalar.dma_start(xlo[:], xlo_s)
                nc.gpsimd.dma_start(wt0[:, 3072:5120], W1p[:, 3072:5120])
                nc.scalar.dma_start(xr[:], xr_s)
                nc.gpsimd.dma_start(wt0[:, 5120:GW], W1p[:, 5120:GW])
                wts[(0, 0)] = wt0
                for i in range(6):
                    nc.tensor.matmul(trash[0:1, 0:128 - i],
                                     c1eps[0:1, 0:1], wrow[0:1, 0:128 - i],
                                     start=True, stop=True)

                for L in range(2):
                    # stats half A = tiles [0, SPLIT), B = the rest; the B
                    # group is the per-layer critical tail so it is smallest
                    # where the epilogue is cheapest (L2: single tile)
                    SPLIT = 6 if L == 0 else 7
                    WA, WB = SPLIT, NT - SPLIT
                    m8, mlo, mr = ((x8, xlo, xr), (h8, h832, hr))[L]
                    srow = (sqk1[0:1, 0:2 * BS], hsqrow[0:1, :])[L]
                    e_all = bigp.tile([128, NT * BS], F32, tag="e")
                    stats = rowp.tile([128, 16], F32, tag="stats")
                    s_t = finp.tile([128, 8], F32, tag="s_t")
                    u_t = finp.tile([128, 8], F32, tag="u_t")
                    gaths = {}

                    def bn_kick(half):
                        mark(f"L{L}-kick{half}")
                        off, w = (0, 2 * WA) if half == 0 else (2 * WA, 2 * WB)
                        cin = dramp.tile([128, w], F32, tag=f"cin{half}")
                        nc.sync.dma_start(cin[:], stats[:, off:off + w])
                        cout = dramp.tile([N_CORES * 128, w], F32,
                                          tag=f"cout{half}",
                                          addr_space="Shared")
                        if collectives:
                            nc.gpsimd.collective_compute(
                                "AllGather", Alu.bypass,
                                replica_groups=[list(range(N_CORES))],
                                ins=[cin[:].opt()], outs=[cout[:].opt()])
                        else:
                            nc.sync.dma_start(cout[0:128, :], cin[:])
                        gath = rowp.tile([128, N_CORES * w], F32,
                                         tag=f"gath{half}", bufs=2)
                        nc.sync.dma_start(
                            gath[:].rearrange("p (g f) -> p g f", g=N_CORES),
                            cout[:].rearrange("(g p) f -> p g f", p=128))
                        gaths[half] = gath

                    def bn_finalize(half):
                        mark(f"L{L}-fin{half}")
                        # s = g * exp(-0.5*ln(var+eps)), u = b - s*mean
                        w = (WA, WB)[half]
                        t0 = (0, WA)[half]
                        red = finp.tile([128, 2 * w], F32, tag=f"red{half}",
                                        bufs=2)
                        nc.vector.tensor_reduce(
                            out=red[:],
                            in_=gaths[half][:].rearrange(
                                "p (g f) -> p f g", g=N_CORES),
                            axis=mybir.AxisListType.X, op=Alu.add)
                        hs = slice(t0, t0 + w)
                        m_e = finp.tile([128, w], F32, tag=f"m_e{half}",
                                        bufs=2)
                        nc.vector.tensor_scalar(out=m_e[:], in0=red[:, 0:w],
                                                scalar1=1.0 / B, scalar2=None,
                                                op0=Alu.mult)
                        mz = finp.tile([128, w], F32, tag=f"mz{half}", bufs=2)
                        nc.vector.tensor_scalar(out=mz[:], in0=red[:, 0:w],
                                                scalar1=1.0 / B, scalar2=-1.0,
                                                op0=Alu.mult, op1=Alu.add)
                        mz2 = finp.tile([128, w], F32, tag=f"mz2{half}",
                                        bufs=2)
                        nc.vector.tensor_tensor(out=mz2[:], in0=mz[:],
                                                in1=mz[:], op=Alu.mult)
                        ams = finp.tile([128, w], F32, tag=f"ams{half}",
                                        bufs=2)
                        nc.vector.tensor_tensor(out=ams[:], in0=red[:, w:2 * w],
                                                in1=red[:, 0:w],
                                                op=Alu.subtract)
                        varr = finp.tile([128, w], F32, tag=f"varr{half}",
                                         bufs=2)
                        nc.vector.scalar_tensor_tensor(
                            out=varr[:], in0=ams[:], scalar=1.0 / B,
                            in1=mz2[:], op0=Alu.mult, op1=Alu.subtract)
                        lnw = finp.tile([128, w], F32, tag=f"lnw{half}",
                                        bufs=2)
                        nc.scalar.activation(lnw[:], varr[:], Act.Ln,
                                             bias=c1eps[:])
                        rs = finp.tile([128, w], F32, tag=f"rs{half}", bufs=2)
                        nc.scalar.activation(rs[:], lnw[:], Act.Exp,
                                             scale=-0.5)
                        gc = 16 * L + t0
                        nc.vector.tensor_tensor(out=s_t[:, hs], in0=rs[:],
                                                in1=gb[:, gc:gc + w],
                                                op=Alu.mult)
                        um = finp.tile([128, w], F32, tag=f"um{half}", bufs=2)
                        nc.vector.tensor_tensor(out=um[:], in0=s_t[:, hs],
                                                in1=m_e[:], op=Alu.mult)
                        nc.vector.tensor_tensor(out=u_t[:, hs],
                                                in0=gb[:, gc + 8:gc + 8 + w],
                                                in1=um[:], op=Alu.subtract)

                    def convert_tile(j):
                        mark(f"conv{j}")
                        # L1 epilogue: h8/h832/hr fp8 splits; |h|^2 matmuls
                        # deferred to flush_hsq. Pool is avoided entirely so
                        # the tail collective (which occupies the Pool queue)
                        # cannot block the converts.
                        js = slice(j * BS, (j + 1) * BS)
                        hh = scr.tile([128, BS], FP8, tag="hh", bufs=8)
                        nc.scalar.activation(hh[:], e_all[:, js], Act.Square,
                                             scale=s_t[:, j:j + 1],
                                             bias=u_t[:, j:j + 1])
                        hh_pend.append((j, hh))
                        hf = scr.tile([128, BS], F32, tag="hf", bufs=4)
                        nc.vector.tensor_scalar(
                            out=hf[:], in0=e_all[:, js],
                            scalar1=s_t[:, j:j + 1], scalar2=u_t[:, j:j + 1],
                            op0=Alu.mult, op1=Alu.add)
                        nc.scalar.copy(h8[:, js], hf[:])
                        nc.vector.tensor_scalar(
                            out=h832[:, js], in0=hf[:],
                            scalar1=1.0 / 32.0, scalar2=None, op0=Alu.mult)
                        nc.vector.tensor_tensor(out=hr[:, js], in0=hf[:],
                                                in1=h8[:, js],
                                                op=Alu.subtract)

                    def flush_hsq():
                        mark("flush_hsq")
                        # deferred |h|^2 matmuls + hi/lo row split, emitted on
                        # the PE queue after L2-j0's k-groups so PE order never
                        # blocks on the late converts
                        for i, (j, hh) in enumerate(hh_pend):
                            nc.tensor.matmul(hsqp[:], onesc[:], hh[:],
                                             start=(i == 0),
                                             stop=(i == len(hh_pend) - 1))
                        hh_pend.clear()
                        nc.scalar.copy(hsqrow[0:1, 0:BS], hsqp[:])
                        hd = rowp.tile([1, BS], F32, tag="hd")
                        nc.vector.tensor_tensor(out=hd[:], in0=hsqp[:],
                                                in1=hsqrow[0:1, 0:BS],
                                                op=Alu.subtract)
                        nc.vector.tensor_scalar(
                            out=hsqrow[0:1, BS:2 * BS], in0=hd[:],
                            scalar1=16.0, scalar2=None, op0=Alu.mult)

                    def store_tile(j):
                        mark(f"store{j}")
                        # L2 epilogue: normalize + residual in place in e_all.
                        # Tiles 0..5 run under the half-B collective (which
                        # occupies Pool), so they stay on DVE; 6,7 may use Pool.
                        js = slice(j * BS, (j + 1) * BS)
                        nc.vector.tensor_scalar(
                            out=e_all[:, js], in0=e_all[:, js],
                            scalar1=s_t[:, j:j + 1], scalar2=u_t[:, j:j + 1],
                            op0=Alu.mult, op1=Alu.add)
                        nc.vector.tensor_tensor(out=e_all[:, js],
                                                in0=e_all[:, js],
                                                in1=xres[:, js], op=Alu.add)

                    def drain(t0, t1, eng):
                        mark(f"drain{t0}")
                        eng.dma_start(
                            outT[t0 * 128:t1 * 128, :].rearrange(
                                "(t p) b -> p t b", p=128),
                            e_all[:, t0 * BS:t1 * BS].rearrange(
                                "p (t b) -> p t b", t=t1 - t0))

                    def sweep_tile(j):
                        mark(f"L{L}-sweep{j}")
                        nxt = None
                        if L == 0:
                            nxt = (0, j + 1) if j < NT - 1 else (1, 0)
                        elif j < NT - 2:
                            nxt = (1, j + 2)
                        if nxt is not None and nxt not in wts:
                            load_wt(*nxt)
                        wt = wts.pop((L, j))
                        # k-groups first: 3 recovery passes per k. For L2-j0,
                        # chunk pairs 0,1 (h tiles 0-3) are emitted before
                        # pairs 2,3 across all passes so PE has ready work
                        # during the BN-B collective gap.
                        split_pairs = (L == 1 and j == 0)
                        crs = ([(0, 1), (2,), (3,)] if split_pairs
                               else [(0, 1, 2, 3)])
                        ps = []
                        for _k in range(K):
                            p = pp.tile([128, BS], F32, tag="p")
                            ps.append(p)
                        started = [False] * K
                        for cr in crs:
                            for k in range(K):
                                w8 = (1 + 2 * k) * 1024
                                passes = [(w8, m8, False),
                                          (w8 + 1024, mlo, False),
                                          (w8, mr, True)]
                                for wb, mv, is_last in passes:
                                    for c in cr:
                                        last = (is_last and c == CH // 2 - 1)
                                        nc.tensor.matmul(
                                            ps[k][:],
                                            dr(wt[:, wb + c * 256:
                                                  wb + (c + 1) * 256]),
                                            dr(mv[:, c * 2 * BS:
                                                  (c + 1) * 2 * BS]),
                                            start=not started[k], stop=last,
                                            perf_mode=DR)
                                        started[k] = True
                        sqk = []
                        for k in range(K):
                            sk = scr.tile([128, BS], F32, tag="sq", bufs=8)
                            ci = L * 32 + j * 4 + k
                            nc.scalar.activation(sk[:], ps[k][:], Act.Square,
                                                 scale=1.0 / 16.0,
                                                 bias=cv[:, ci:ci + 1])
                            sqk.append(sk)
                        # combine chain entirely on DVE: no cross-engine hops
                        s01 = scr.tile([128, BS], F32, tag="s01")
                        nc.vector.tensor_tensor(out=s01[:], in0=sqk[0][:],
                                                in1=sqk[1][:], op=Alu.add)
                        s23 = scr.tile([128, BS], F32, tag="s23")
                        nc.vector.tensor_tensor(out=s23[:], in0=sqk[2][:],
                                                in1=sqk[3][:], op=Alu.add)
                        # q-group last: mu part + fused hi/lo sq rows
                        if L == 1 and j == 0:
                            flush_hsq()
                        mark(f"L{L}-q{j}")
                        q = pq.tile([128, BS], F32, tag="q")
                        for c in range(CH // 2):
                            nc.tensor.matmul(
                                q[:], dr(wt[:, c * 256:(c + 1) * 256]),
                                dr(m8[:, c * 2 * BS:(c + 1) * 2 * BS]),
                                start=(c == 0), stop=False, perf_mode=DR)
                        nc.tensor.matmul(q[:], dr(sqk1[0:1, 1024:1280]),
                                         dr(srow), start=False, stop=True,
                                         perf_mode=DR)
                        t1 = scr.tile([128, BS], F32, tag="t1")
                        nc.vector.scalar_tensor_tensor(
                            out=t1[:], in0=q[:],
                            scalar=lamc[:, L * 8 + j: L * 8 + j + 1],
                            in1=s01[:], op0=Alu.mult, op1=Alu.add)
                        qf = scr.tile([128, BS], F32, tag="qf")
                        nc.vector.tensor_tensor(out=qf[:], in0=t1[:],
                                                in1=s23[:], op=Alu.add)
                        ej = e_all[:, j * BS:(j + 1) * BS]
                        s1c = j if j < SPLIT else 2 * WA + (j - SPLIT)
                        nc.scalar.activation(ej, qf[:], Act.Exp,
                                             scale=-1.0 / D,
                                             bias=cexp[:, L * 8 + j: L * 8 + j + 1],
                                             accum_out=stats[:, s1c:s1c + 1])
                        # A_j = sum(e^2 - e); var recovered in finalize
                        ac = WA + j if j < SPLIT else 2 * WA + WB + (j - SPLIT)
                        scrq = scr.tile([128, BS], F32, tag="scrq")
                        nc.vector.scalar_tensor_tensor(
                            out=scrq[:], in0=ej, scalar=-1.0, in1=ej,
                            op0=Alu.add, op1=Alu.mult,
                            accum_out=stats[:, ac:ac + 1])

                    post = convert_tile if L == 0 else store_tile
                    for j in range(NT):
                        sweep_tile(j)
                        if j == SPLIT - 1:
                            bn_kick(0)
                    if L == 0:
                        load_wt(1, 1)
                    bn_kick(1)
                    if L == 0:
                        # PE keep-warm chain across the collective gap
                        for i in range(7):
                            nc.tensor.matmul(
                                trash[:], c1eps[0:1, 0:1],
                                e_all[0:1, (i % NT) * BS:(i % NT) * BS + BS],
                                start=True, stop=True)
                        load_wt(1, 2)
                        nc.gpsimd.dma_start(xres[:], xres_s)
                    bn_finalize(0)
                    for t in range(SPLIT):
                        post(t)
                        if L == 1 and t == 2:
                            drain(0, 3, nc.scalar)
                        elif L == 1 and t == 5:
                            drain(3, 6, nc.scalar)
                        elif L == 1 and t == 6:
                            drain(6, 7, nc.scalar)
                    bn_finalize(1)
                    for t in range(SPLIT, NT):
                        post(t)
                        if L == 1:
                            drain(t, t + 1, nc.scalar)

            if loop_reps:
                with tc.For_i(0, loop_reps, 1):
                    body()
            else:
                for _rep in range(reps):
                    body()

    nc.compile()
    return nc


def _f8(a):
    return np.asarray(a, np.float32).astype(F8)


def _host_prep(x, mu1, lam1, v1, g1, b1, mu2, lam2, v2, g2, b2):
    """Build the device-input arrays (fp8 weights/activations, f32 consts)."""
    def chunkify(blk):
        # [D, 128] column block -> [128, CH*128] chunk-major partition layout
        return blk.reshape(CH, 128, 128).transpose(1, 0, 2).reshape(128, CH * 128)

    def pack_layer(mu, lam_, v, c_shift, x_shift):
        mu64 = mu.astype(np.float64)
        v64 = v.astype(np.float64)
        lam64 = lam_.astype(np.float64)
        Wmu = (-2.0 * mu64).T                              # [D, N] (no lam)
        vs = 16.0 * v64.transpose(1, 0, 2).reshape(K * N, D).T   # [D, K*N]
        v8 = _f8(vs)
        vr8 = _f8((vs - v8.astype(np.float64)) * 32.0)
        blocks = []
        for j in range(NT):
            cols = slice(j * 128, (j + 1) * 128)
            blocks.append(_f8(chunkify(Wmu[:, cols])))
            for k in range(K):
                kc = slice(k * N + j * 128, k * N + (j + 1) * 128)
                blocks.append(chunkify(v8[:, kc]))
                blocks.append(chunkify(vr8[:, kc]))
        Wp = np.concatenate(blocks, axis=1)                # [128, WCOL] fp8
        # constants: proj = v.(x'+x_shift) - v.mu ; -2mu.(x'+x_shift)
        vm = (v64 * mu64[:, None, :]).sum(-1)              # [N, K]
        vsh = v64.sum(-1) * x_shift                        # [N, K]
        cv_l = (vsh - vm).reshape(NT, 128, K).transpose(1, 0, 2)
        musq = (mu64 * mu64).sum(1)
        mush = mu64.sum(1) * (2.0 * x_shift)
        ce = (-(lam64 * (musq + c_shift - mush)) / D).reshape(NT, 128).T
        lc = lam64.reshape(NT, 128).T
        return (Wp, cv_l.reshape(128, NT * K).astype(np.float32),
                ce.astype(np.float32), lc.astype(np.float32))

    W1pk, cv1, ce1, lc1 = pack_layer(mu1, lam1, v1, C1, 0.5)
    W2pk, cv2, ce2, lc2 = pack_layer(mu2, lam2, v2, 0.0, 0.0)
    cv_all = np.concatenate([cv1, cv2], axis=1)               # [128, 64]
    cexp = np.concatenate([ce1, ce2], axis=1)                 # [128, 16]
    lamc = np.concatenate([lc1, lc2], axis=1)                 # [128, 16]
    gbp = np.concatenate(
        [a.reshape(NT, 128).T for a in (g1, b1, g2, b2)],
        axis=1).astype(np.float32)                            # [128, 32]
    cst = np.concatenate([lamc, cexp, cv_all, gbp],
                         axis=1).astype(np.float32)           # [128, 128]
    k1 = np.concatenate([np.ones(128), np.full(128, 1.0 / 16.0)])

    x64 = x.astype(np.float64)
    xT = np.ascontiguousarray(x.T)                            # [D, B]
    sqv = (x64 * x64).sum(1) - C1                             # [B]
    sq8 = _f8(sqv)
    sqlo = _f8((sqv - sq8.astype(np.float64)) * 16.0)

    in_maps = []
    for c in range(N_CORES):
        rs = slice(c * BS, (c + 1) * BS)
        xte = (xT[:, rs].reshape(CH, 128, BS).transpose(1, 0, 2)
               .reshape(128, CH * BS))
        x8 = _f8(xte - 0.5)
        xlo = _f8(x8.astype(np.float32) / 32.0)
        xr = _f8(xte - 0.5 - x8.astype(np.float64))
        sqk1 = np.concatenate(
            [sq8[rs], sqlo[rs], _f8(k1)]).reshape(1, 2 * BS + 256)
        in_maps.append({
            "x8_s": x8, "xlo_s": xlo, "xr_s": xr,
            "xres_s": xte.astype(BF),
            "sqk1_s": sqk1,
            "W1p": W1pk, "W2p": W2pk,
            "cst_s": cst,
        })
    return in_maps


def kernel(x, mu1, lam1, v1, g1, b1, mu2, lam2, v2, g2, b2):
    if "nc" not in _CACHE:
        _CACHE["nc"] = _build_nc()
    nc = _CACHE["nc"]
    in_maps = _host_prep(x, mu1, lam1, v1, g1, b1, mu2, lam2, v2, g2, b2)
    res = run_bass_kernel_spmd(nc, in_maps, list(range(N_CORES)))
    return np.concatenate(
        [res.results[c]["outT"].T for c in range(N_CORES)], axis=0)


# revision 19
# speedup vs baseline: 5.0724x; 5.0724x over previous
"""Trainium2 Bass kernel for nn_BasicBlock_HMU (two HMU layers + sync BN + residual).

Sharding: data-parallel over batch (8 cores x 512 rows); params replicated.
BN batch statistics are all-gathered across the 8 cores (sync BN).

v4 — builds on v3.1 (n-on-partitions, fp8e4m3 DoubleRow, full error
compensation) with:
  - pass-3 weight reuse: the third recovery pass v832*(m-m8)*32 is replaced
    by v8*fp8(m-m8) — identical accuracy in emulation (6.28e-3 vs 6.24e-3)
    and drops the third weight stream: 9 blocks/n-tile instead of 13
    (18.8MB instead of 27.2MB of weights per core).
  - split-half sync BN: stats for n-tiles 0-3 are AllGathered at mid-sweep
    and finalized/normalized while tiles 4-7 still sweep; only the second
    half's collective + finalize sits on the critical path.
  - rsqrt via exp(-0.5*ln(w)): Ln/Exp/Square/Copy all live in one ACT
    table set, so no LoadActFuncSet thrash (Sqrt's set excludes Exp).
  - k-groups first, q-group last per n-tile: shortens the last-tile
    combine tail (t1=lam*q+s01 -> qf -> exp) and moves the |h|^2-row
    dependency off the L2 start; hsq matmuls for tiles 4-7 are emitted
    after L2-j0's k-groups so PE order never blocks on late converts.
  - L2 normalize+residual in place in e_all, drained in 3 merged DMAs.
  - startup: per-k first-tile weight DMAs interleaved with the x streams;
    distinct PE warm-up matmuls hand off to the sweep at full clock.
"""

import numpy as np
import ml_dtypes

import concourse.bacc as bacc
import concourse.mybir as mybir
import concourse.tile as tile

try:
    from concourse.bass_utils import run_bass_kernel_spmd
except ImportError:  # pragma: no cover
    from bass_utils import run_bass_kernel_spmd

# All ACT functions this kernel uses (Exp, Square, Ln, Copy/Identity) live in
# the single table set natural_log_exp_and_others. The default chooser mixes
# sets (Exp/Square from one, Ln from another), inserting 1.3us
# LoadActFuncSet switches on the critical path. Restrict it to the one set
# that covers everything (positions preserved so act_func_set_id still
# indexes act_info.json).
_ACT_SET = "natural_log_exp_and_others"
_orig_gat = bacc.get_activation_tables


def _gat_single(arch):
    tabs = _orig_gat(arch)
    if _ACT_SET in tabs:
        tabs = {k: (v if k == _ACT_SET else set()) for k, v in tabs.items()}
    return tabs


bacc.get_activation_tables = _gat_single

F32 = mybir.dt.float32
BF16 = mybir.dt.bfloat16
FP8 = mybir.dt.float8e4
Alu = mybir.AluOpType
Act = mybir.ActivationFunctionType
DR = mybir.MatmulPerfMode.DoubleRow
BF = ml_dtypes.bfloat16
F8 = mybir.dt.np(mybir.dt.float8e4)

N_CORES = 8
B, D, N, K = 4096, 1024, 1024, 4
BS = B // N_CORES          # 512 rows per core
NT = N // 128              # 8 n-tiles per layer
CH = D // 128              # 8 contraction chunks (4 DoubleRow pairs)
NBLK = 1 + 2 * K           # mu + (v8, vr8) x4 blocks per n-tile group
GW = NBLK * 1024           # packed group width (9216)
WCOL = NT * GW             # packed weight columns per layer
BN_EPS = 1e-5
C1 = 1024.0 / 3.0          # host-side shift of the |x|^2 row

_CACHE = {}
_MARKERS = []


def _build_nc(reps=1, loop_reps=0, collectives=True):
    nc = bacc.Bacc("TRN2", target_bir_lowering=False, debug=False,
                   num_devices=N_CORES)
    _MARKERS.clear()

    def mark(label):
        _MARKERS.append((label, nc._state.next_id()))

    x8_s = nc.dram_tensor("x8_s", [128, CH * BS], FP8, kind="ExternalInput").ap()
    xlo_s = nc.dram_tensor("xlo_s", [128, CH * BS], FP8, kind="ExternalInput").ap()
    xr_s = nc.dram_tensor("xr_s", [128, CH * BS], FP8, kind="ExternalInput").ap()
    xres_s = nc.dram_tensor("xres_s", [128, CH * BS], BF16, kind="ExternalInput").ap()
    sqk1_s = nc.dram_tensor("sqk1_s", [1, 2 * BS + 256], FP8,
                            kind="ExternalInput").ap()
    W1p = nc.dram_tensor("W1p", [128, WCOL], FP8, kind="ExternalInput").ap()
    W2p = nc.dram_tensor("W2p", [128, WCOL], FP8, kind="ExternalInput").ap()
    cst_s = nc.dram_tensor("cst_s", [128, 128], F32, kind="ExternalInput").ap()
    outT = nc.dram_tensor("outT", [N, BS], F32, kind="ExternalOutput").ap()

    def dr(ap):
        return ap.rearrange("p (two m) -> p two m", two=2)

    with tile.TileContext(nc) as tc:
        with (
            tc.tile_pool(name="const", bufs=1) as constp,
            tc.tile_pool(name="big", bufs=1) as bigp,
            tc.tile_pool(name="wp", bufs=4) as wp,
            tc.tile_pool(name="scr", bufs=2) as scr,
            tc.tile_pool(name="rowp", bufs=1) as rowp,
            tc.tile_pool(name="fin", bufs=2) as finp,
            tc.tile_pool(name="pq", bufs=2, space="PSUM") as pq,
            tc.tile_pool(name="pp", bufs=4, space="PSUM") as pp,
            tc.tile_pool(name="ph", bufs=1, space="PSUM") as php,
            tc.tile_pool(name="dram", bufs=2, space="DRAM") as dramp,
        ):
            # ---- constants (loaded once, shared across reps) ----
            sqk1 = constp.tile([1, 2 * BS + 256], FP8)
            nc.scalar.dma_start(sqk1[:], sqk1_s)
            cst = constp.tile([128, 128], F32)
            nc.scalar.dma_start(cst[:], cst_s)
            lamc = cst[:, 0:16]
            cexp = cst[:, 16:32]
            cv = cst[:, 32:96]
            gb = cst[:, 96:128]
            onesc = constp.tile([128, 1], FP8)
            nc.gpsimd.memset(onesc[:], 1.0)
            c1eps = constp.tile([128, 1], F32)
            nc.gpsimd.memset(c1eps[:], 1.0 + BN_EPS)
            wrow = constp.tile([1, 128], F32)
            nc.gpsimd.memset(wrow[:], 1.0)

            def body():
                x8 = bigp.tile([128, CH * BS], FP8, tag="x8")
                xlo = bigp.tile([128, CH * BS], FP8, tag="xlo")
                xr = bigp.tile([128, CH * BS], FP8, tag="xr")
                xres = bigp.tile([128, CH * BS], BF16, tag="xres")
                h8 = bigp.tile([128, NT * BS], FP8, tag="h8")
                h832 = bigp.tile([128, NT * BS], FP8, tag="h832")
                hr = bigp.tile([128, NT * BS], FP8, tag="hr")
                hsqrow = rowp.tile([1, 2 * BS], FP8, tag="hsqrow")
                trash = php.tile([1, BS], F32, tag="trash")
                hsqp = php.tile([1, BS], F32, tag="hsq")

                wts = {}
                hh_pend = []

                def load_wt(L, j):
                    wt = wp.tile([128, GW], FP8, tag="w")
                    Wp = (W1p, W2p)[L]
                    b0 = j * GW
                    nc.sync.dma_start(wt[:, 0:1024], Wp[:, b0:b0 + 1024])
                    q = (nc.gpsimd, nc.scalar)[j % 2]
                    q.dma_start(wt[:, 1024:GW], Wp[:, b0 + 1024:b0 + GW])
                    wts[(L, j)] = wt

                # ---- startup: first tile's weights split per-k and
                # interleaved with the x streams so k0 can start earliest ----
                wt0 = wp.tile([128, GW], FP8, tag="w")
                nc.sync.dma_start(wt0[:, 0:1024], W1p[:, 0:1024])
                nc.scalar.dma_start(x8[:, 0:4 * BS], x8_s[:, 0:4 * BS])
                nc.gpsimd.dma_start(wt0[:, 1024:3072], W1p[:, 1024:3072])
                nc.scalar.dma_start(x8[:, 4 * BS:CH * BS],
                                    x8_s[:, 4 * BS:CH * BS])
                nc.scalar.dma_start(xlo[:], xlo_s)
                nc.gpsimd.dma_start(wt0[:, 3072:5120], W1p[:, 3072:5120])
                nc.scalar.dma_start(xr[:], xr_s)
                nc.gpsimd.dma_start(wt0[:, 5120:GW], W1p[:, 5120:GW])
                wts[(0, 0)] = wt0
                for i in range(6):
                    nc.tensor.matmul(trash[0:1, 0:128 - i],
                                     c1eps[0:1, 0:1], wrow[0:1, 0:128 - i],
                                     start=True, stop=True)

                for L in range(2):
                    # stats half A = tiles [0, SPLIT), B = the rest; the B
                    # group is the per-layer critical tail so it is smallest
                    # where the epilogue is cheapest (L2: single tile)
                    SPLIT = 6 if L == 0 else 7
                    WA, WB = SPLIT, NT - SPLIT
                    m8, mlo, mr = ((x8, xlo, xr), (h8, h832, hr))[L]
                    srow = (sqk1[0:1, 0:2 * BS], hsqrow[0:1, :])[L]
                    e_all = bigp.tile([128, NT * BS], F32, tag="e")
                    stats = rowp.tile([128, 16], F32, tag="stats")
                    s_t = finp.tile([128, 8], F32, tag="s_t")
                    u_t = finp.tile([128, 8], F32, tag="u_t")
                    gaths = {}

                    def bn_kick(half):
                        mark(f"L{L}-kick{half}")
                        off, w = (0, 2 * WA) if half == 0 else (2 * WA, 2 * WB)
                        cin = dramp.tile([128, w], F32, tag=f"cin{half}")
                        nc.sync.dma_start(cin[:], stats[:, off:off + w])
                        cout = dramp.tile([N_CORES * 128, w], F32,
                                          tag=f"cout{half}",
                                          addr_space="Shared")
                        if collectives:
                            nc.gpsimd.collective_compute(
                                "AllGather", Alu.bypass,
                                replica_groups=[list(range(N_CORES))],
                                ins=[cin[:].opt()], outs=[cout[:].opt()])
                        else:
                            nc.sync.dma_start(cout[0:128, :], cin[:])
                        gath = rowp.tile([128, N_CORES * w], F32,
                                         tag=f"gath{half}", bufs=2)
                        nc.sync.dma_start(
                            gath[:].rearrange("p (g f) -> p g f", g=N_CORES),
                            cout[:].rearrange("(g p) f -> p g f", p=128))
                        gaths[half] = gath

                    def bn_finalize(half):
                        mark(f"L{L}-fin{half}")
                        # s = g * exp(-0.5*ln(var+eps)), u = b - s*mean
                        w = (WA, WB)[half]
                        t0 = (0, WA)[half]
                        red = finp.tile([128, 2 * w], F32, tag=f"red{half}",
                                        bufs=2)
                        nc.vector.tensor_reduce(
                            out=red[:],
                            in_=gaths[half][:].rearrange(
                                "p (g f) -> p f g", g=N_CORES),
                            axis=mybir.AxisListType.X, op=Alu.add)
                        hs = slice(t0, t0 + w)
                        m_e = finp.tile([128, w], F32, tag=f"m_e{half}",
                                        bufs=2)
                        nc.vector.tensor_scalar(out=m_e[:], in0=red[:, 0:w],
                                                scalar1=1.0 / B, scalar2=None,
                                                op0=Alu.mult)
                        mz = finp.tile([128, w], F32, tag=f"mz{half}", bufs=2)
                        nc.vector.tensor_scalar(out=mz[:], in0=red[:, 0:w],
                                                scalar1=1.0 / B, scalar2=-1.0,
                                                op0=Alu.mult, op1=Alu.add)
                        mz2 = finp.tile([128, w], F32, tag=f"mz2{half}",
                                        bufs=2)
                        nc.vector.tensor_tensor(out=mz2[:], in0=mz[:],
                                                in1=mz[:], op=Alu.mult)
                        ams = finp.tile([128, w], F32, tag=f"ams{half}",
                                        bufs=2)
                        nc.vector.tensor_tensor(out=ams[:], in0=red[:, w:2 * w],
                                                in1=red[:, 0:w],
                                                op=Alu.subtract)
                        varr = finp.tile([128, w], F32, tag=f"varr{half}",
                                         bufs=2)
                        nc.vector.scalar_tensor_tensor(
                            out=varr[:], in0=ams[:], scalar=1.0 / B,
                            in1=mz2[:], op0=Alu.mult, op1=Alu.subtract)
                        lnw = finp.tile([128, w], F32, tag=f"lnw{half}",
                                        bufs=2)
                        nc.scalar.activation(lnw[:], varr[:], Act.Ln,
                                             bias=c1eps[:])
                        rs = finp.tile([128, w], F32, tag=f"rs{half}", bufs=2)
                        nc.scalar.activation(rs[:], lnw[:], Act.Exp,
                                             scale=-0.5)
                        gc = 16 * L + t0
                        nc.vector.tensor_tensor(out=s_t[:, hs], in0=rs[:],
                                                in1=gb[:, gc:gc + w],
                                                op=Alu.mult)
                        um = finp.tile([128, w], F32, tag=f"um{half}", bufs=2)
                        nc.vector.tensor_tensor(out=um[:], in0=s_t[:, hs],
                                                in1=m_e[:], op=Alu.mult)
                        nc.vector.tensor_tensor(out=u_t[:, hs],
                                                in0=gb[:, gc + 8:gc + 8 + w],
                                                in1=um[:], op=Alu.subtract)

                    def convert_tile(j):
                        mark(f"conv{j}")
                        # L1 epilogue: h8/h832/hr fp8 splits; |h|^2 matmuls
                        # deferred to flush_hsq. Pool is avoided entirely so
                        # the tail collective (which occupies the Pool queue)
                        # cannot block the converts.
                        js = slice(j * BS, (j + 1) * BS)
                        hh = scr.tile([128, BS], FP8, tag="hh", bufs=8)
                        nc.scalar.activation(hh[:], e_all[:, js], Act.Square,
                                             scale=s_t[:, j:j + 1],
                                             bias=u_t[:, j:j + 1])
                        hh_pend.append((j, hh))
                        hf = scr.tile([128, BS], F32, tag="hf", bufs=4)
                        nc.vector.tensor_scalar(
                            out=hf[:], in0=e_all[:, js],
                            scalar1=s_t[:, j:j + 1], scalar2=u_t[:, j:j + 1],
                            op0=Alu.mult, op1=Alu.add)
                        nc.scalar.copy(h8[:, js], hf[:])
                        nc.vector.tensor_scalar(
                            out=h832[:, js], in0=hf[:],
                            scalar1=1.0 / 32.0, scalar2=None, op0=Alu.mult)
                        nc.vector.tensor_tensor(out=hr[:, js], in0=hf[:],
                                                in1=h8[:, js],
                                                op=Alu.subtract)

                    def flush_hsq():
                        mark("flush_hsq")
                        # deferred |h|^2 matmuls + hi/lo row split, emitted on
                        # the PE queue after L2-j0's k-groups so PE order never
                        # blocks on the late converts
                        for i, (j, hh) in enumerate(hh_pend):
                            nc.tensor.matmul(hsqp[:], onesc[:], hh[:],
                                             start=(i == 0),
                                             stop=(i == len(hh_pend) - 1))
                        hh_pend.clear()
                        nc.scalar.copy(hsqrow[0:1, 0:BS], hsqp[:])
                        hd = rowp.tile([1, BS], F32, tag="hd")
                        nc.vector.tensor_tensor(out=hd[:], in0=hsqp[:],
                                                in1=hsqrow[0:1, 0:BS],
                                                op=Alu.subtract)
                        nc.vector.tensor_scalar(
                            out=hsqrow[0:1, BS:2 * BS], in0=hd[:],
                            scalar1=16.0, scalar2=None, op0=Alu.mult)

                    def store_tile(j):
                        mark(f"store{j}")
                        # L2 epilogue: normalize + residual in place in e_all.
                        # Tiles 0..5 run under the half-B collective (which
                        # occupies Pool), so they stay on DVE; 6,7 may use Pool.
                        js = slice(j * BS, (j + 1) * BS)
                        nc.vector.tensor_scalar(
                            out=e_all[:, js], in0=e_all[:, js],
                            scalar1=s_t[:, j:j + 1], scalar2=u_t[:, j:j + 1],
                            op0=Alu.mult, op1=Alu.add)
                        nc.vector.tensor_tensor(out=e_all[:, js],
                                                in0=e_all[:, js],
                                                in1=xres[:, js], op=Alu.add)

                    def drain(t0, t1, eng):
                        mark(f"drain{t0}")
                        eng.dma_start(
                            outT[t0 * 128:t1 * 128, :].rearrange(
                                "(t p) b -> p t b", p=128),
                            e_all[:, t0 * BS:t1 * BS].rearrange(
                                "p (t b) -> p t b", t=t1 - t0))

                    def sweep_tile(j):
                        mark(f"L{L}-sweep{j}")
                        nxt = None
                        if L == 0:
                            nxt = (0, j + 1) if j < NT - 1 else (1, 0)
                        elif j < NT - 2:
                            nxt = (1, j + 2)
                        if nxt is not None and nxt not in wts:
                            load_wt(*nxt)
                        wt = wts.pop((L, j))
                        # k-groups first: 3 recovery passes per k. For L2-j0,
                        # chunk pairs 0,1 (h tiles 0-3) are emitted before
                        # pairs 2,3 across all passes so PE has ready work
                        # during the BN-B collective gap.
                        split_pairs = (L == 1 and j == 0)
                        crs = ([(0, 1), (2,), (3,)] if split_pairs
                               else [(0, 1, 2, 3)])
                        ps = []
                        for _k in range(K):
                            p = pp.tile([128, BS], F32, tag="p")
                            ps.append(p)
                        started = [False] * K
                        for cr in crs:
                            for k in range(K):
                                w8 = (1 + 2 * k) * 1024
                                passes = [(w8, m8, False),
                                          (w8 + 1024, mlo, False),
                                          (w8, mr, True)]
                                for wb, mv, is_last in passes:
                                    for c in cr:
                                        last = (is_last and c == CH // 2 - 1)
                                        nc.tensor.matmul(
                                            ps[k][:],
                                            dr(wt[:, wb + c * 256:
                                                  wb + (c + 1) * 256]),
                                            dr(mv[:, c * 2 * BS:
                                                  (c + 1) * 2 * BS]),
                                            start=not started[k], stop=last,
                                            perf_mode=DR)
                                        started[k] = True
                        sqk = []
                        for k in range(K):
                            sk = scr.tile([128, BS], F32, tag="sq", bufs=8)
                            ci = L * 32 + j * 4 + k
                            nc.scalar.activation(sk[:], ps[k][:], Act.Square,
                                                 scale=1.0 / 16.0,
                                                 bias=cv[:, ci:ci + 1])
                            sqk.append(sk)
                        # combine chain entirely on DVE: no cross-engine hops
                        s01 = scr.tile([128, BS], F32, tag="s01")
                        nc.vector.tensor_tensor(out=s01[:], in0=sqk[0][:],
                                                in1=sqk[1][:], op=Alu.add)
                        s23 = scr.tile([128, BS], F32, tag="s23")
                        nc.vector.tensor_tensor(out=s23[:], in0=sqk[2][:],
                                                in1=sqk[3][:], op=Alu.add)
                        # q-group last: mu part + fused hi/lo sq rows
                        if L == 1 and j == 0:
                            flush_hsq()
                        mark(f"L{L}-q{j}")
                        q = pq.tile([128, BS], F32, tag="q")
                        for c in range(CH // 2):
                            nc.tensor.matmul(
                                q[:], dr(wt[:, c * 256:(c + 1) * 256]),
                                dr(m8[:, c * 2 * BS:(c + 1) * 2 * BS]),
                                start=(c == 0), stop=False, perf_mode=DR)
                        nc.tensor.matmul(q[:], dr(sqk1[0:1, 1024:1280]),
                                         dr(srow), start=False, stop=True,
                                         perf_mode=DR)
                        t1 = scr.tile([128, BS], F32, tag="t1")
                        nc.vector.scalar_tensor_tensor(
                            out=t1[:], in0=q[:],
                            scalar=lamc[:, L * 8 + j: L * 8 + j + 1],
                            in1=s01[:], op0=Alu.mult, op1=Alu.add)
                        qf = scr.tile([128, BS], F32, tag="qf")
                        nc.vector.tensor_tensor(out=qf[:], in0=t1[:],
                                                in1=s23[:], op=Alu.add)
                        ej = e_all[:, j * BS:(j + 1) * BS]
                        s1c = j if j < SPLIT else 2 * WA + (j - SPLIT)
                        nc.scalar.activation(ej, qf[:], Act.Exp,
                                             scale=-1.0 / D,
                                             bias=cexp[:, L * 8 + j: L * 8 + j + 1],
                                             accum_out=stats[:, s1c:s1c + 1])
                        # A_j = sum(e^2 - e); var recovered in finalize
                        ac = WA + j if j < SPLIT else 2 * WA + WB + (j - SPLIT)
                        scrq = scr.tile([128, BS], F32, tag="scrq")
                        nc.vector.scalar_tensor_tensor(
                            out=scrq[:], in0=ej, scalar=-1.0, in1=ej,
                            op0=Alu.add, op1=Alu.mult,
                            accum_out=stats[:, ac:ac + 1])

                    post = convert_tile if L == 0 else store_tile
                    for j in range(NT):
                        sweep_tile(j)
                        if j == SPLIT - 1:
                            bn_kick(0)
                    if L == 0:
                        load_wt(1, 1)
                    bn_kick(1)
                    if L == 0:
                        # PE keep-warm chain across the collective gap
                        for i in range(9):
                            nc.tensor.matmul(
                                trash[:], c1eps[0:1, 0:1],
                                e_all[0:1, (i % NT) * BS:(i % NT) * BS + BS],
                                start=True, stop=True)
                        load_wt(1, 2)
                        nc.gpsimd.dma_start(xres[:], xres_s)
                    bn_finalize(0)
                    for t in range(SPLIT):
                        post(t)
                        if L == 1 and t == 2:
                            drain(0, 3, nc.scalar)
                        elif L == 1 and t == 5:
                            drain(3, 6, nc.scalar)
                        elif L == 1 and t == 6:
                            drain(6, 7, nc.scalar)
                    bn_finalize(1)
                    for t in range(SPLIT, NT):
                        post(t)
                        if L == 1:
                            drain(t, t + 1, nc.scalar)

            if loop_reps:
                with tc.For_i(0, loop_reps, 1):
                    body()
            else:
                for _rep in range(reps):
                    body()

    nc.compile()
    return nc


def _f8(a):
    return np.asarray(a, np.float32).astype(F8)


def _host_prep(x, mu1, lam1, v1, g1, b1, mu2, lam2, v2, g2, b2):
    """Build the device-input arrays (fp8 weights/activations, f32 consts)."""
    def chunkify(blk):
        # [D, 128] column block -> [128, CH*128] chunk-major partition layout
        return blk.reshape(CH, 128, 128).transpose(1, 0, 2).reshape(128, CH * 128)

    def pack_layer(mu, lam_, v, c_shift, x_shift):
        mu64 = mu.astype(np.float64)
        v64 = v.astype(np.float64)
        lam64 = lam_.astype(np.float64)
        Wmu = (-2.0 * mu64).T                              # [D, N] (no lam)
        vs = 16.0 * v64.transpose(1, 0, 2).reshape(K * N, D).T   # [D, K*N]
        v8 = _f8(vs)
        vr8 = _f8((vs - v8.astype(np.float64)) * 32.0)
        blocks = []
        for j in range(NT):
            cols = slice(j * 128, (j + 1) * 128)
            blocks.append(_f8(chunkify(Wmu[:, cols])))
            for k in range(K):
                kc = slice(k * N + j * 128, k * N + (j + 1) * 128)
                blocks.append(chunkify(v8[:, kc]))
                blocks.append(chunkify(vr8[:, kc]))
        Wp = np.concatenate(blocks, axis=1)                # [128, WCOL] fp8
        # constants: proj = v.(x'+x_shift) - v.mu ; -2mu.(x'+x_shift)
        vm = (v64 * mu64[:, None, :]).sum(-1)              # [N, K]
        vsh = v64.sum(-1) * x_shift                        # [N, K]
        cv_l = (vsh - vm).reshape(NT, 128, K).transpose(1, 0, 2)
        musq = (mu64 * mu64).sum(1)
        mush = mu64.sum(1) * (2.0 * x_shift)
        ce = (-(lam64 * (musq + c_shift - mush)) / D).reshape(NT, 128).T
        lc = lam64.reshape(NT, 128).T
        return (Wp, cv_l.reshape(128, NT * K).astype(np.float32),
                ce.astype(np.float32), lc.astype(np.float32))

    W1pk, cv1, ce1, lc1 = pack_layer(mu1, lam1, v1, C1, 0.5)
    W2pk, cv2, ce2, lc2 = pack_layer(mu2, lam2, v2, 0.0, 0.0)
    cv_all = np.concatenate([cv1, cv2], axis=1)               # [128, 64]
    cexp = np.concatenate([ce1, ce2], axis=1)                 # [128, 16]
    lamc = np.concatenate([lc1, lc2], axis=1)                 # [128, 16]
    gbp = np.concatenate(
        [a.reshape(NT, 128).T for a in (g1, b1, g2, b2)],
        axis=1).astype(np.float32)                            # [128, 32]
    cst = np.concatenate([lamc, cexp, cv_all, gbp],
                         axis=1).astype(np.float32)           # [128, 128]
    k1 = np.concatenate([np.ones(128), np.full(128, 1.0 / 16.0)])

    x64 = x.astype(np.float64)
    xT = np.ascontiguousarray(x.T)                            # [D, B]
    sqv = (x64 * x64).sum(1) - C1                             # [B]
    sq8 = _f8(sqv)
    sqlo = _f8((sqv - sq8.astype(np.float64)) * 16.0)

    in_maps = []
    for c in range(N_CORES):
        rs = slice(c * BS, (c + 1) * BS)
        xte = (xT[:, rs].reshape(CH, 128, BS).transpose(1, 0, 2)
               .reshape(128, CH * BS))
        x8 = _f8(xte - 0.5)
        xlo = _f8(x8.astype(np.float32) / 32.0)
        xr = _f8(xte - 0.5 - x8.astype(np.float64))
        sqk1 = np.concatenate(
            [sq8[rs], sqlo[rs], _f8(k1)]).reshape(1, 2 * BS + 256)
        in_maps.append({
            "x8_s": x8, "xlo_s": xlo, "xr_s": xr,
            "xres_s": xte.astype(BF),
            "sqk1_s": sqk1,
            "W1p": W1pk, "W2p": W2pk,
            "cst_s": cst,
        })
    return in_maps


def kernel(x, mu1, lam1, v1, g1, b1, mu2, lam2, v2, g2, b2):
    if "nc" not in _CACHE:
        _CACHE["nc"] = _build_nc()
    nc = _CACHE["nc"]
    in_maps = _host_prep(x, mu1, lam1, v1, g1, b1, mu2, lam2, v2, g2, b2)
    res = run_bass_kernel_spmd(nc, in_maps, list(range(N_CORES)))
    return np.concatenate(
        [res.results[c]["outT"].T for c in range(N_CORES)], axis=0)
